# revision 72
# baseline (speedup 1.0000x reference)
"""Cross-attention kernel for Trainium2, 8-core head-sharded (tensor parallel).

Problem: x[2,2048,1024], context[2,2048,768], Wq[1024,1024], Wkv[768,2048],
Wo[1024,1024]; H=16 heads, Dh=64; out = softmax(q k^T / 8) v @ Wo.

Sharding: 2 heads per core (128 q/k/v columns). Each core computes its
heads' attention output projected through its 128-row slice of Wo,
producing a full-shape bf16 partial; host sums the 8 partials in f32.

Per-core dataflow (matmuls bf16 except where noted, fp32 PSUM accum):
  qT[128,4096]  = Wq_slice^T @ x^T        (lhsT = Wq_slice, rhs = xT)
  kT[128,4096]  = Wk_slice^T @ ctx^T
  v[4096,130]   = ctx @ Wv_slice, +ones columns (flash denominator trick)
  per (batch b, head h, n-tile of 512 queries):
    S^T[m,n]    = kT_h^T-slice matmuls  (K=64)  -> PSUM [128, 2xn512]
    expST       = Exp(S^T * 0.125)  on ScalarE -> SBUF (bf16; fp8e4m3
                  for m-tiles 0-3 of each batch)
    OT[.,512]  += v_aug_h^T @ expST  (accumulate over 16 m-tiles;
                  m-tiles 0-3 run as 2 fp8 DoubleRow passes - the
                  quarter-fp8 fraction adds ~0.9% rel err, measured
                  total 0.0133 vs the 0.02 gate)
                  row 64 = softmax denominator
    OT_norm     = OT[0:64] * bcast(1/denom)   (DVE/Pool + gpsimd broadcast)
  out[n,1024]   = OTcomb^T @ Wo_slice  -> bf16 partial, DMA to DRAM

Scheduling (PE busy ~87% in TimelineSim):
- The S->exp->O chain is software-pipelined one group deep: group g+1's
  S matmuls are emitted before group g's O matmuls, so the ScalarE exp
  latency is structurally hidden from the PE stream.
- The attention inner loop is ACT(exp)-bound per group, so projections
  (q/k/v) and the output projection are chopped into ~0.5us "filler"
  parts popped into the PE stream at the per-group stall points. The
  final-out for tile t is deferred 2-3 tiles so the last tiles (which
  have no projection work left) still get PE filler.
- PSUM (8 banks): pst 2x2 (S double-buffer) + pot 2 (O accumulators) +
  mixp/pof 1+1 (projection/final staging, alternated so consecutive
  parts don't serialize on the PSUM->SBUF copy).
- Tail: only the last tile's out-projection is exposed; it reads head B
  pre-shift (K=64 split against a restaged Wo half) so the partition-
  shift DMA is off the critical path, alternates two PSUM slots and
  both DVE+ACT for copies, and 3 ready parts are reserved to fill the
  normalize-latency window.
"""

from collections import deque

import numpy as np
import ml_dtypes

import concourse.bass as bass
import concourse.mybir as mybir
import concourse.tile as tile
from concourse import bacc
from concourse.bass_utils import run_bass_kernel_spmd

BF16 = mybir.dt.bfloat16
FP8 = mybir.dt.float8e4
F32 = mybir.dt.float32
NPBF16 = ml_dtypes.bfloat16

B, N, C = 2, 2048, 1024
M = 2048          # context length
CTX = 768
H = 16
DH = 64
NCORES = 8
HEADS_PER_CORE = H // NCORES          # 2
HC = HEADS_PER_CORE * DH              # 128 columns per core
ROWS = B * N                          # 4096 flattened rows
SCALE = DH ** -0.5                    # 0.125

KQ = C // 128                         # 8 k-tiles for q projection
KC = CTX // 128                       # 6 k-tiles for kv projections
NT = 512                              # query-tile width (free dim)
MT = 128                              # m-tile (context block = partitions)
N_NT = N // NT                        # 4 query tiles per batch
N_MT = M // MT                        # 16 m-tiles per batch
MG = 2                                # m-tiles per exp group ([128,1024] ACT calls)
N_G = N_MT // MG                      # 8 groups per tile

_CACHE = {}


class FillerQueue:
    """PE-work parts popped into the attention stream's stall points."""

    def __init__(self):
        self.q = deque()

    def add(self, fn, cost):
        self.q.append((fn, cost))

    def pop_budget(self, budget):
        spent = 0
        while self.q and (spent == 0 or spent + self.q[0][1] <= budget):
            fn, c = self.q.popleft()
            fn()
            spent += c

    def drain(self):
        while self.q:
            fn, _ = self.q.popleft()
            fn()


def build_kernel(repeat=1):
    """Build and compile the per-core Bass module (same program, all cores)."""
    nc = bacc.Bacc(None)

    xT_d = nc.dram_tensor("xT", [C, ROWS], BF16, kind="ExternalInput")
    cT_d = nc.dram_tensor("ctxT", [CTX, ROWS], BF16, kind="ExternalInput")
    wq_d = nc.dram_tensor("wq", [C, HC], BF16, kind="ExternalInput")
    wk_d = nc.dram_tensor("wk", [CTX, HC], BF16, kind="ExternalInput")
    wv_d = nc.dram_tensor("wv", [CTX, HC], BF16, kind="ExternalInput")
    wo_d = nc.dram_tensor("wo", [HC, C], BF16, kind="ExternalInput")
    out_d = nc.dram_tensor("out", [ROWS, C], BF16, kind="ExternalOutput")

    with tile.TileContext(nc) as tc:
        with (
            tc.tile_pool(name="const", bufs=1) as const,
            tc.tile_pool(name="ctx_res", bufs=1) as ctx_res,
            tc.tile_pool(name="act_res", bufs=1) as act_res,
            tc.tile_pool(name="xstream", bufs=4) as xstream,
            tc.tile_pool(name="expp", bufs=6) as expp,
            tc.tile_pool(name="otcomb", bufs=3) as otcomb_p,
            tc.tile_pool(name="nrm", bufs=4) as nrm,
            tc.tile_pool(name="ostage", bufs=3) as ostage,
            tc.tile_pool(name="pst", bufs=2, space="PSUM") as pst,
            tc.tile_pool(name="pot", bufs=2, space="PSUM") as pot,
            tc.tile_pool(name="mixp", bufs=1, space="PSUM") as mixp,
            tc.tile_pool(name="pof", bufs=1, space="PSUM") as pof,
        ):
          for _rep in range(repeat):
            wq_sb = const.tile([128, KQ, HC], BF16, tag="wq")
            nc.sync.dma_start(out=wq_sb, in_=wq_d.rearrange("(t p) m -> p t m", p=128))
            wk_sb = const.tile([128, KC, HC], BF16, tag="wk")
            wv_sb = const.tile([128, KC, HC], BF16, tag="wv")
            wo_sb = const.tile([128, C], BF16, tag="wo")
            wo_lo = const.tile([64, C], BF16, tag="wolo")

            ctx_sb = ctx_res.tile([128, KC, ROWS], BF16, tag="ctx")
            cT_pm = cT_d.rearrange("(t p) n -> p t n", p=128)   # [128,6,4096]
            xT_pm = xT_d.rearrange("(t p) n -> p t n", p=128)   # [128,8,4096]

            kT_sb = act_res.tile([128, ROWS], BF16, tag="kT")
            vago = act_res.tile([128, ROWS // 128, 130], BF16, tag="vaug")
            nc.vector.memset(vago[:, :, 64], 1.0)
            nc.vector.memset(vago[:, :, 129], 1.0)
            # fp8 copy of v (+ones col 64, zero pad) for the two groups per
            # tile that run the O accumulation in fp8 DoubleRow; quartering
            # the fp8 fraction keeps the added error at ~sqrt(1/4) of the
            # full-fp8 variant (which measured 2.6% > the 2% gate)
            vago8 = act_res.tile([128, 8, 2, 128], FP8, tag="vaug8")
            nc.vector.memset(vago8[:, :, :, 64], 1.0)
            nc.vector.memset(vago8[:, :, :, 65:128], 0.0)
            qT_sb = act_res.tile([128, ROWS], BF16, tag="qT")

            xs_tiles = {}

            def qt_prefetch(n, split=False):
                xs = xstream.tile([128, KQ, NT], BF16, tag="xs")
                if split:
                    # two half-DMAs so the first matmuls can start sooner
                    nc.sync.dma_start(out=xs[:, 0:KQ // 2, :],
                                      in_=xT_pm[:, 0:KQ // 2, bass.ts(n, NT)])
                    nc.sync.dma_start(out=xs[:, KQ // 2:KQ, :],
                                      in_=xT_pm[:, KQ // 2:KQ, bass.ts(n, NT)])
                else:
                    nc.sync.dma_start(out=xs, in_=xT_pm[:, :, bass.ts(n, NT)])
                xs_tiles[n] = xs

            def ctx_quarter(n):
                nc.sync.dma_start(out=ctx_sb[:, :, bass.ts(n, NT)],
                                  in_=cT_pm[:, :, bass.ts(n, NT)])

            def qt_chunk_inline(n):
                xs = xs_tiles.pop(n)
                psq = mixp.tile([128, NT], F32, tag="psq")
                for t in range(KQ):
                    nc.tensor.matmul(psq, wq_sb[:, t, :], xs[:, t, :],
                                     start=(t == 0), stop=(t == KQ - 1))
                nc.vector.tensor_copy(qT_sb[:, bass.ts(n, NT)], psq)

            def add_qt_parts(fq, n):
                st = {}

                def mk(t0):
                    def run():
                        if t0 == 0:
                            st["xs"] = xs_tiles.pop(n)
                            st["ps"] = mixp.tile([128, NT], F32, tag="psq", name="psq")
                        for t in (t0, t0 + 1):
                            nc.tensor.matmul(st["ps"], wq_sb[:, t, :],
                                             st["xs"][:, t, :],
                                             start=(t == 0), stop=(t == KQ - 1))
                        if t0 + 2 == KQ:
                            nc.vector.tensor_copy(qT_sb[:, bass.ts(n, NT)], st["ps"])
                    return run

                for t0 in range(0, KQ, 2):
                    fq.add(mk(t0), 440)

            def kv_chunk_inline(c):
                ps = mixp.tile([128, NT], F32, tag="psq")
                for t in range(KC):
                    nc.tensor.matmul(ps, wk_sb[:, t, :],
                                     ctx_sb[:, t, bass.ts(c, NT)],
                                     start=(t == 0), stop=(t == KC - 1))
                nc.vector.tensor_copy(kT_sb[:, bass.ts(c, NT)], ps)
                for mm in range(NT // 128):
                    m = c * 4 + mm
                    psv = (mixp if mm % 2 == 0 else pof).tile(
                        [128, NT], F32,
                        tag="psq" if mm % 2 == 0 else "fp", name="psv")
                    for t in range(KC):
                        nc.tensor.matmul(psv[:, 0:HC],
                                         ctx_sb[:, t, bass.ts(m, 128)],
                                         wv_sb[:, t, :],
                                         start=(t == 0), stop=(t == KC - 1))
                    nc.vector.tensor_copy(vago[:, m, 0:64], psv[:, 0:64])
                    nc.vector.tensor_copy(vago[:, m, 65:129], psv[:, 64:128])
                    if m % 16 < 4:
                        m8 = m % 16 + 4 * (m // 16)
                        nc.vector.tensor_copy(vago8[:, m8, 0, 0:64], psv[:, 0:64])
                        nc.vector.tensor_copy(vago8[:, m8, 1, 0:64], psv[:, 64:128])

            def add_kv_parts(fq, c):
                st = {}

                def mk_k(t0):
                    def run():
                        if t0 == 0:
                            st["ps"] = mixp.tile([128, NT], F32, tag="psq", name="psq")
                        for t in (t0, t0 + 1):
                            nc.tensor.matmul(st["ps"], wk_sb[:, t, :],
                                             ctx_sb[:, t, bass.ts(c, NT)],
                                             start=(t == 0), stop=(t == KC - 1))
                        if t0 + 2 == KC:
                            nc.vector.tensor_copy(kT_sb[:, bass.ts(c, NT)], st["ps"])
                    return run

                for t0 in range(0, KC, 2):
                    fq.add(mk_k(t0), 440)

                def mk_v(mm):
                    def run():
                        m = c * 4 + mm
                        # alternate PSUM slots so consecutive v-parts don't
                        # serialize on the previous part's DVE copy
                        psv = (mixp if mm % 2 == 0 else pof).tile(
                            [128, NT], F32,
                            tag="psq" if mm % 2 == 0 else "fp", name="psv")
                        for t in range(KC):
                            nc.tensor.matmul(psv[:, 0:HC],
                                             ctx_sb[:, t, bass.ts(m, 128)],
                                             wv_sb[:, t, :],
                                             start=(t == 0), stop=(t == KC - 1))
                        nc.vector.tensor_copy(vago[:, m, 0:64], psv[:, 0:64])
                        nc.vector.tensor_copy(vago[:, m, 65:129], psv[:, 64:128])
                        if m % 16 < 4:
                            m8 = m % 16 + 4 * (m // 16)
                            nc.vector.tensor_copy(vago8[:, m8, 0, 0:64], psv[:, 0:64])
                            nc.vector.tensor_copy(vago8[:, m8, 1, 0:64], psv[:, 64:128])
                    return run

                for mm in range(4):
                    fq.add(mk_v(mm), 380)

            def add_final_parts(fq, b, nt, otc, two_slot=False, otn=None):
                st = {}

                def mk(s, cpart):
                    def run():
                        if cpart == 0:
                            st[s] = ostage.tile([128, C], BF16, tag="ost", name="ost")
                        even = (2 * s + cpart) % 2 == 0
                        if two_slot:
                            # latency-bound: alternate 2 PSUM slots so the
                            # next matmul overlaps the previous copy
                            fp = (pof.tile([128, NT], F32, tag="fp", name="fp")
                                  if even else
                                  mixp.tile([128, NT], F32, tag="psq", name="fp"))
                        else:
                            fp = pof.tile([128, NT], F32, tag="fp", name="fp")
                        if otn is None:
                            nc.tensor.matmul(fp, otc[:, bass.ts(s, 128)],
                                             wo_sb[:, bass.ts(cpart, NT)],
                                             start=True, stop=True)
                        else:
                            # tail: head B taken pre-shift from otn so the
                            # partition-shift DMA is off the critical path;
                            # wo_lo stages Wo rows 64:128 at partitions 0:64
                            nc.tensor.matmul(fp, otc[0:64, bass.ts(s, 128)],
                                             wo_sb[0:64, bass.ts(cpart, NT)],
                                             start=True, stop=False)
                            nc.tensor.matmul(fp, otn[:, bass.ts(s, 128)],
                                             wo_lo[:, bass.ts(cpart, NT)],
                                             start=False, stop=True)
                        if otn is not None and not even:
                            # ACT is idle in the tail: split copies across
                            # both engines so the copy chain halves
                            nc.scalar.copy(st[s][:, bass.ts(cpart, NT)], fp)
                        else:
                            nc.vector.tensor_copy(st[s][:, bass.ts(cpart, NT)], fp)
                        if otn is not None:
                            nc.sync.dma_start(
                                out=out_d[bass.ds(b * N + nt * NT + s * 128, 128),
                                          bass.ts(cpart, NT)],
                                in_=st[s][:, bass.ts(cpart, NT)])
                        elif cpart == 1:
                            nc.sync.dma_start(
                                out=out_d[bass.ds(b * N + nt * NT + s * 128, 128), :],
                                in_=st[s])
                    return run

                for s in range(NT // 128):
                    for cpart in range(C // NT):
                        fq.add(mk(s, cpart), 440)

            def emit_S(b, nt, g, fq=None, budget=450):
                """Both heads' S^T matmuls + exp for group g; returns exps."""
                nsl = bass.ds(b * N + nt * NT, NT)
                exps = []
                for h in range(2):
                    hd = bass.ds(h * DH, DH)
                    st_ps = pst.tile([128, MG, NT], F32, tag="st")
                    exp_sb = expp.tile([128, MG, NT], BF16 if g >= 2 else FP8,
                                       tag="exp", name="exp")
                    for j in range(MG):
                        mt = g * MG + j
                        msl = bass.ds(b * M + mt * MT, MT)
                        nc.tensor.matmul(st_ps[:, j, :],
                                         kT_sb[hd, msl], qT_sb[hd, nsl],
                                         start=True, stop=True)
                    nc.scalar.activation(
                        exp_sb, st_ps,
                        mybir.ActivationFunctionType.Exp, scale=SCALE)
                    exps.append(exp_sb)
                if fq is not None:
                    fq.pop_budget(budget)
                return exps

            def emit_O(b, nt, g, exps, ot, fq, otc=None, last=False,
                       budget=450, norm_halves=1):
                """Both heads' O accumulation for group g (one-group lag
                behind emit_S, so exp latency is structurally hidden)."""
                for h, ot_ps in enumerate(ot):
                    if g < 2:
                        # fp8 DoubleRow: both m-tiles of the group in one
                        # pass; group 0 is the accumulation starter so all
                        # 128 PSUM rows are initialized
                        nc.tensor.matmul(
                            ot_ps,
                            vago8[:, bass.ds(b * 4 + g * MG, MG), h, :],
                            exps[h],
                            start=(g == 0), stop=False,
                            perf_mode=mybir.MatmulPerfMode.DoubleRow)
                        if last and h == 0 and otc is not None:
                            normalize_head(0, ot[0], otc, halves=norm_halves)
                        if fq is not None and not last:
                            fq.pop_budget(budget)
                        continue
                    vsl = bass.ds(h * 65, 65)
                    for j in range(MG):
                        mt = g * MG + j
                        nc.tensor.matmul(
                            ot_ps[0:65, :],
                            vago[:, (b * M) // 128 + mt, vsl],
                            exps[h][:, j, :],
                            start=False, stop=(mt == N_MT - 1))
                    if last and h == 0 and otc is not None:
                        normalize_head(0, ot[0], otc, halves=norm_halves)
                    if fq is not None and not last:
                        # defer the last group's pops past the tile boundary
                        # so the DVE queue is clear for the normalize muls
                        fq.pop_budget(budget)

            def normalize_head(h, ot_ps, otc, shift=True, halves=1):
                """softmax-normalize one head's O^T into otc (bf16).

                halves=2 pipelines the recip/broadcast/mul chain in column
                halves so downstream consumers of the first half start
                earlier (used on the latency-bound last tile)."""
                otn = None
                if h == 1:
                    otn = nrm.tile([64, NT], BF16, tag="otn")
                hw_ = NT // halves
                for i in range(halves):
                    csl = bass.ds(i * hw_, hw_)
                    rec = nrm.tile([1, NT], F32, tag="rec", name="rec")
                    nc.vector.reciprocal(rec[:, 0:hw_], ot_ps[64:65, csl])
                    bc = nrm.tile([64, NT], F32, tag="bc", name="bc")
                    nc.gpsimd.partition_broadcast(bc[:, 0:hw_], rec[:, 0:hw_])
                    if h == 0:
                        nc.vector.tensor_mul(otc[0:64, csl], ot_ps[0:64, csl],
                                             bc[:, 0:hw_])
                    else:
                        nc.vector.tensor_mul(otn[:, csl], ot_ps[0:64, csl],
                                             bc[:, 0:hw_])
                        if shift:
                            # partition shift 0:64 -> 64:128 via SBUF DMA
                            nc.sync.dma_start(out=otc[64:128, csl],
                                              in_=otn[:, csl])
                return otn

            # ================= schedule =================
            fqc = FillerQueue()   # constrained parts: drain by tile end
            fqf = FillerQueue()   # final-out parts: may spill across tiles
            fqt = FillerQueue()   # parts reserved for the tail's normalize window

            class BothQ:
                def pop_budget(self, budget):
                    if fqc.q:
                        fqc.pop_budget(budget)
                    else:
                        fqf.pop_budget(budget)

            bq = BothQ()

            # ---- preamble + tile (0,0): projections JIT ----
            qt_prefetch(0, split=True)
            qt_chunk_inline(0)
            nc.sync.dma_start(out=wk_sb, in_=wk_d.rearrange("(t p) m -> p t m", p=128))
            ctx_quarter(0)
            nc.sync.dma_start(
                out=wv_sb, in_=wv_d.rearrange("(t p) m -> p t m", p=128))
            ot_A = pot.tile([128, NT], F32, tag="ot")
            ot_B = pot.tile([128, NT], F32, tag="ot")
            ot = (ot_A, ot_B)
            otc = otcomb_p.tile([128, NT], BF16, tag="otc")

            kv_chunk_inline(0)
            pend = None           # (g, exps) O-block one group behind S
            for c in range(4):
                if c == 0:
                    ctx_quarter(1)
                    nc.sync.dma_start(out=wo_sb, in_=wo_d[:])
                    qt_prefetch(1)
                elif c == 1:
                    ctx_quarter(2)
                    qt_prefetch(2)
                elif c == 2:
                    ctx_quarter(3)
                    add_qt_parts(fqc, 1)
                else:
                    ctx_quarter(4)
                if c < 3:
                    add_kv_parts(fqc, c + 1)
                for g in (2 * c, 2 * c + 1):
                    exps = emit_S(0, 0, g, bq, budget=1500)
                    if pend is not None:
                        emit_O(0, 0, pend[0], pend[1], ot, bq, budget=1500)
                    pend = (g, exps)
                if c < 3:
                    fqc.drain()   # kv chunk c+1 complete before pair c+1
            emit_O(0, 0, pend[0], pend[1], ot, bq, otc=otc, last=True,
                   budget=1500)
            fqc.drain()
            normalize_head(1, ot[1], otc)
            otcs = {(0, 0): otc}

            # ---- remaining tiles ----
            plan = [
                # (b, nt, pre-dma, kv chunk, qt chunk, finals)
                (0, 1, [lambda: ctx_quarter(5), lambda: qt_prefetch(3)], 4, 2, []),
                (0, 2, [lambda: ctx_quarter(6), lambda: qt_prefetch(4)], 5, 3, [(0, 0)]),
                (0, 3, [lambda: ctx_quarter(7), lambda: qt_prefetch(5)], 6, 4, [(0, 1)]),
                (1, 0, [lambda: qt_prefetch(6)], 7, 5, [(0, 2)]),
                (1, 1, [lambda: qt_prefetch(7),
                        lambda: nc.sync.dma_start(out=wo_lo, in_=wo_d[64:128, :])],
                 None, 6, [(0, 3)]),
                (1, 2, [], None, 7, [(1, 0)]),
                (1, 3, [], None, None, [(1, 1)]),
            ]
            for b, nt, pre, kvc, qtc, fins in plan:
                for p in pre:
                    p()
                if kvc is not None:
                    add_kv_parts(fqc, kvc)
                if qtc is not None:
                    add_qt_parts(fqc, qtc)
                for f in fins:
                    # two-slot fp mode is safe when no projection parts
                    # share the mixp pool in this tile
                    add_final_parts(fqf, f[0], f[1], otcs.pop(f),
                                    two_slot=(kvc is None and qtc is None))
                if (b, nt) == (1, 3):
                    # finals (1,2): most parts pop mid-tile; 3 are reserved
                    # to fill the tail's normalize latency window
                    add_final_parts(fqf, 1, 2, otcs.pop((1, 2)), two_slot=True)
                    moved = [fqf.q.pop() for _ in range(3)][::-1]
                    fqt.q.extend(moved)
                ot_A = pot.tile([128, NT], F32, tag="ot")
                ot_B = pot.tile([128, NT], F32, tag="ot")
                ot = (ot_A, ot_B)
                otc = otcomb_p.tile([128, NT], BF16, tag="otc")
                pend = None
                for g in range(N_G):
                    exps = emit_S(b, nt, g, bq)
                    if pend is not None:
                        emit_O(b, nt, pend[0], pend[1], ot, bq)
                    pend = (g, exps)
                is_last = (b, nt) == (1, 3)
                emit_O(b, nt, pend[0], pend[1], ot, bq, otc=otc, last=True)
                fqc.drain()
                otn = normalize_head(1, ot[1], otc, shift=not is_last)
                otcs[(b, nt)] = otc

            # ---- tail: last tile's out-projection ----
            fqf.drain()
            fqt.drain()   # reserved ready parts overlap the normalize chain
            add_final_parts(fqf, 1, 3, otcs.pop((1, 3)), two_slot=True, otn=otn)
            fqf.drain()

    nc.compile()
    return nc


def _shard_inputs(x, context, Wq, Wkv, Wo):
    xf = np.ascontiguousarray(x.reshape(ROWS, C).T).astype(NPBF16)
    cf = np.ascontiguousarray(context.reshape(ROWS, CTX).T).astype(NPBF16)
    in_maps = []
    for c in range(NCORES):
        hc = slice(HC * c, HC * (c + 1))
        in_maps.append({
            "xT": xf,
            "ctxT": cf,
            "wq": np.ascontiguousarray(Wq[:, hc]).astype(NPBF16),
            "wk": np.ascontiguousarray(Wkv[:, hc]).astype(NPBF16),
            "wv": np.ascontiguousarray(Wkv[:, C + HC * c:C + HC * (c + 1)]).astype(NPBF16),
            "wo": np.ascontiguousarray(Wo[hc, :]).astype(NPBF16),
        })
    return in_maps


def get_nc():
    if "nc" not in _CACHE:
        _CACHE["nc"] = build_kernel()
    return _CACHE["nc"]


def run_cores(in_maps, **kw):
    nc = get_nc()
    return run_bass_kernel_spmd(nc, in_maps, list(range(NCORES)), **kw)


def kernel(x, context, Wq, Wkv, Wo):
    in_maps = _shard_inputs(
        np.asarray(x, np.float32), np.asarray(context, np.float32),
        np.asarray(Wq, np.float32), np.asarray(Wkv, np.float32),
        np.asarray(Wo, np.float32))
    res = run_cores(in_maps)
    acc = np.asarray(res.results[0]["out"], dtype=np.float32)
    for i in range(1, NCORES):
        acc = acc + np.asarray(res.results[i]["out"], dtype=np.float32)
    return acc.reshape(B, N, C)
